# revision 16
# baseline (speedup 1.0000x reference)
"""Block-sparse position-wise FFN on Trainium2 (Bass/Tile), 8-core data-parallel.

Strategy:
  - Shard tokens (B*S = 36928) evenly across 8 cores: 4616 tokens/core.
    The FFN is pointwise over tokens and both (masked) weight matrices fit
    in SBUF, so data-parallel needs no collectives.
  - Host prep (free for HW time): apply the 8x8 block masks to W1/W2,
    pre-transpose all operands to the layouts the PE wants. x is
    host-transposed to [DIM, T] so the device never transposes anything.
  - Both layers stream TOKENS as the moving operand with fp32r weights
    stationary (fp32r matmuls self-load weights — no LDWEIGHTS bubble;
    1 cycle/column at N >= 256). Outputs are produced transposed
    ([feature, token]) and un-transposed on host:
      fc1: hT[m]   = gelu(W1mT[:,m].T @ xT + b1[m])   per 128-wide f-tile
      fc2: outT[g] = W2mT[:,g].T @ hT + b2[g]         per 128-wide d-tile
    PE work per 256-token segment: (24*6 + 6*24) matmuls of N=256
    at ~109 ns each; ACT does gelu+bias (fc1) and copy+bias (fc2).
"""

import sys
import types

import numpy as np
import ml_dtypes

# concourse's axon trace path imports antenv.axon_hooks, which this image
# lacks; install a no-op shim so an env-requested trace degrades gracefully
# instead of raising ImportError.
try:
    import antenv.axon_hooks  # noqa: F401
except ImportError:
    import antenv

    _hooks = types.ModuleType("antenv.axon_hooks")
    _hooks._hook = None
    _hooks.set_axon_ntff_profile_hook = (
        lambda h: setattr(_hooks, "_hook", h))
    _hooks.get_axon_ntff_profile_hook = lambda: _hooks._hook
    sys.modules["antenv.axon_hooks"] = _hooks
    antenv.axon_hooks = _hooks

import concourse.bass as bass
import concourse.bacc as bacc
import concourse.mybir as mybir
from concourse import tile
from concourse.bass_utils import run_bass_kernel_spmd

B, S, DIM, FF, BLK = 64, 577, 768, 3072, 8
NCORES = 8
TOK = B * S                # 36928
T = TOK // NCORES          # 4616 tokens per core
P = 128
KD = DIM // P              # 6 d-tiles
KF = FF // P               # 24 f-tiles
SEG = 384                  # tokens per segment (fp32r full rate needs >=256)
F32 = mybir.dt.float32
F32R = mybir.dt.float32r
BF16 = mybir.dt.bfloat16
BF16NP = ml_dtypes.bfloat16
GELU = mybir.ActivationFunctionType.Gelu
IDENT = mybir.ActivationFunctionType.Identity


def _segments(total):
    """256-token segments; a trailing remainder <256 is folded into the last
    segment so every segment stays in [256, 512] (fp32r full rate, one PSUM
    bank)."""
    out, pos = [], 0
    while pos < total:
        rem = total - pos
        w = SEG if rem > 2 * SEG else rem
        out.append((pos, w))
        pos += w
    return out


SEGW = 392  # max segment width ([384]*11 + [392] for T=4616)


def _body(tc, x_d, w1_d, b1_d, w2_d, b2_d, o_d, t_tokens, K1, K2):
    nc = tc.nc
    with (
        tc.tile_pool(name="const", bufs=1) as constp,
        tc.tile_pool(name="wpool", bufs=1) as wp,
        tc.tile_pool(name="xt", bufs=2) as xtp,
        tc.tile_pool(name="ht", bufs=24) as htp,
        tc.tile_pool(name="wstage", bufs=2) as wstp,
        tc.tile_pool(name="onat", bufs=3) as onatp,
        tc.tile_pool(name="ps1", bufs=3, space=bass.MemorySpace.PSUM) as ps1p,
        tc.tile_pool(name="ps2", bufs=3, space=bass.MemorySpace.PSUM) as ps2p,
    ):
        b1_s = constp.tile([P, KF], F32)
        nc.sync.dma_start(out=b1_s[:], in_=b1_d)
        b2_s = constp.tile([P, KD], F32)
        nc.sync.dma_start(out=b2_s[:], in_=b2_d)

        # fc1 weights: w1_s[k] = W1mT[k*128:(k+1)*128, :]  ([128 d, 3072 f])
        # loaded in 4 column chunks so fc1 can start after the first chunk.
        w1_s = [wp.tile([P, FF], F32R, tag=f"w1_{k}", name=f"w1_{k}")
                for k in range(KD)]

        def load_cvt(dst, src_ap, cols):
            # weights ship as bf16: full-rate HW-DGE DMA into a small staging
            # tile, then the (otherwise idle) DVE widens to fp32r in SBUF.
            st = wstp.tile([P, 384], BF16, tag="wst")
            nc.sync.dma_start(out=st[:, 0:cols], in_=src_ap)
            nc.vector.tensor_copy(dst, st[:, 0:cols])

        W1CHUNK = 384
        for cc in range(FF // W1CHUNK):
            for k in range(KD):
                load_cvt(
                    w1_s[k][:, cc * W1CHUNK:(cc + 1) * W1CHUNK],
                    w1_d[k * P:(k + 1) * P,
                         cc * W1CHUNK:(cc + 1) * W1CHUNK],
                    W1CHUNK,
                )
        # fc2 weights: w2_s[k] = W2mT[k*128:(k+1)*128, :]  ([128 f, 768 d])
        w2_s = []
        for k in range(KF):
            w = wp.tile([P, DIM], F32R, tag=f"w2_{k}")
            for hh in range(2):
                load_cvt(
                    w[:, hh * 384:(hh + 1) * 384],
                    w2_d[k * P:(k + 1) * P, hh * 384:(hh + 1) * 384],
                    384,
                )
            w2_s.append(w)

        for (s0, w) in _segments(t_tokens):
            # x arrives pre-transposed from host: xT tile [128 d, w tokens]
            xts = []
            for k in range(KD):
                xt = xtp.tile([P, SEGW], F32R, tag=f"xt{k}", name=f"xt{k}")
                nc.gpsimd.dma_start(
                    out=xt[:, 0:w], in_=x_d[k * P:(k + 1) * P, s0:s0 + w]
                )
                xts.append(xt)

            # --- fc1: hT[m] = gelu(W1mT[:,m].T @ xT + b1[m]) ---
            hts = []
            for m in range(KF):
                ps = ps1p.tile([P, SEGW], F32, tag="ps1")
                ks = K1[m]
                for j, k in enumerate(ks):
                    nc.tensor.matmul(
                        ps[:, 0:w],
                        w1_s[k][:, m * P:(m + 1) * P],
                        xts[k][:, 0:w],
                        start=(j == 0), stop=(j == len(ks) - 1),
                    )
                ht = htp.tile([P, SEGW], F32R, tag="ht")
                nc.scalar.activation(
                    ht[:, 0:w], ps[:, 0:w], GELU, bias=b1_s[:, m:m + 1]
                )
                hts.append(ht)

            # --- fc2: outT[g] = W2mT[:,g].T @ hT + b2[g] ---
            for g in range(KD):
                ps = ps2p.tile([P, SEGW], F32, tag="ps2")
                ks2 = K2[g]
                for j, k in enumerate(ks2):
                    nc.tensor.matmul(
                        ps[:, 0:w],
                        w2_s[k][:, g * P:(g + 1) * P],
                        hts[k][:, 0:w],
                        start=(j == 0), stop=(j == len(ks2) - 1),
                    )
                on = onatp.tile([P, SEGW], F32, tag="on")
                nc.scalar.activation(
                    on[:, 0:w], ps[:, 0:w], IDENT, bias=b2_s[:, g:g + 1]
                )
                nc.sync.dma_start(
                    out=o_d[g * P:(g + 1) * P, s0:s0 + w], in_=on[:, 0:w]
                )


def build_program(K1, K2, t_tokens=T):
    nc = bacc.Bacc("TRN2", target_bir_lowering=False, debug=False,
                   num_devices=NCORES)
    x_d = nc.dram_tensor("xt", [DIM, t_tokens], F32R,
                         kind="ExternalInput").ap()
    w1_d = nc.dram_tensor("w1t", [DIM, FF], BF16, kind="ExternalInput").ap()
    b1_d = nc.dram_tensor("b1", [P, KF], F32, kind="ExternalInput").ap()
    w2_d = nc.dram_tensor("w2t", [FF, DIM], BF16, kind="ExternalInput").ap()
    b2_d = nc.dram_tensor("b2", [P, KD], F32, kind="ExternalInput").ap()
    o_d = nc.dram_tensor("out", [DIM, t_tokens], F32,
                         kind="ExternalOutput").ap()
    with tile.TileContext(nc) as tc:
        _body(tc, x_d, w1_d, b1_d, w2_d, b2_d, o_d, t_tokens, K1, K2)
    nc.compile()
    return nc


def _round_fp32r(a):
    """Round fp32 values to the fp32r grid (low 12 mantissa bits dropped,
    round-to-nearest), matching the PE's fp32r operand rounding."""
    u = a.view(np.uint32)
    u = (u + np.uint32(0x800)) & np.uint32(0xFFFFF000)
    return u.view(np.float32)


def host_prep(x, W1, b1, W2, b2, mask1, mask2, perm_f, perm_d):
    xt = _round_fp32r(np.ascontiguousarray(
        np.asarray(x, dtype=np.float32).reshape(TOK, DIM).T))  # [DIM, TOK]
    m1 = np.repeat(np.repeat(np.asarray(mask1, dtype=bool), BLK, 0), BLK, 1)
    m2 = np.repeat(np.repeat(np.asarray(mask2, dtype=bool), BLK, 0), BLK, 1)
    pf = (perm_f[:, None] * BLK + np.arange(BLK)[None, :]).ravel()
    pd = (perm_d[:, None] * BLK + np.arange(BLK)[None, :]).ravel()
    w1m = np.asarray(W1, np.float32) * m1.astype(np.float32)
    w1t = np.ascontiguousarray(w1m[pf].T).astype(BF16NP)          # [DIM, FFp]
    w2m = np.asarray(W2, np.float32) * m2.astype(np.float32)
    w2t = np.ascontiguousarray(
        w2m[np.ix_(pd, pf)].T).astype(BF16NP)                     # [FFp, DIMp]
    b1h = np.ascontiguousarray(
        np.asarray(b1, np.float32)[pf].reshape(KF, P).T)          # [P, KF]
    b2h = np.ascontiguousarray(
        np.asarray(b2, np.float32)[pd].reshape(KD, P).T)          # [P, KD]
    return xt, w1t, b1h, w2t, b2h, pd


def _cluster(sup, tsz):
    """Greedy: build tiles of `tsz` blocks sharing a commonly-avoided window,
    so that tile's matmul chain can skip that window entirely."""
    nb, nwin = sup.shape
    unassigned = list(range(nb))
    tiles = []
    while True:
        best_w, best_av = None, None
        for wdw in range(nwin):
            av = [f for f in unassigned if not sup[f, wdw]]
            if len(av) >= tsz and (best_av is None or len(av) > len(best_av)):
                best_w, best_av = wdw, av
        if best_w is None:
            break
        best_av.sort(key=lambda f: int((~sup[f]).sum()))
        take = best_av[:tsz]
        tiles.append(take)
        for f in take:
            unassigned.remove(f)
    while unassigned:
        tiles.append(unassigned[:tsz])
        unassigned = unassigned[tsz:]
    perm = np.array([f for t in tiles for f in t])
    K = []
    for t in tiles:
        u = sup[t].any(0)
        ks = [wdw for wdw in range(nwin) if u[wdw]] or [0]
        K.append(ks)
    return perm, K


def _plan(mask1, mask2):
    m1 = np.asarray(mask1, dtype=bool)
    m2 = np.asarray(mask2, dtype=bool)
    sup1 = m1.reshape(FF // BLK, KD, P // BLK).any(2)
    perm_f, K1 = _cluster(sup1, P // BLK)
    sup2 = m2[:, perm_f].reshape(DIM // BLK, KF, P // BLK).any(2)
    perm_d, K2 = _cluster(sup2, P // BLK)
    return perm_f, K1, perm_d, K2


_PROGRAM = None
_PROGRAM_KEY = None


def _get_program(mask1, mask2):
    global _PROGRAM, _PROGRAM_KEY
    key = (np.asarray(mask1).tobytes(), np.asarray(mask2).tobytes())
    if _PROGRAM is None or _PROGRAM_KEY != key:
        perm_f, K1, perm_d, K2 = _plan(mask1, mask2)
        _PROGRAM = (build_program(K1, K2, T), perm_f, perm_d)
        _PROGRAM_KEY = key
    return _PROGRAM


def kernel(x, W1, b1, W2, b2, mask1, mask2, **run_kwargs):
    nc, perm_f, perm_d = _get_program(mask1, mask2)
    xt, w1t, b1h, w2t, b2h, pd = host_prep(
        x, W1, b1, W2, b2, mask1, mask2, perm_f, perm_d)
    in_maps = [
        {"xt": np.ascontiguousarray(xt[:, c * T:(c + 1) * T]),
         "w1t": w1t, "b1": b1h, "w2t": w2t, "b2": b2h}
        for c in range(NCORES)
    ]
    res = run_bass_kernel_spmd(nc, in_maps, list(range(NCORES)), **run_kwargs)
    out = np.concatenate(
        [res.results[c]["out"] for c in range(NCORES)], axis=1)  # [DIMp, TOK]
    inv = np.empty(DIM, np.int64)
    inv[pd] = np.arange(DIM)
    out = out[inv]
    out = np.ascontiguousarray(out.T).reshape(B, S, DIM).astype(np.float32)
    if run_kwargs:
        kernel.last_results = res
    return out


# revision 18
# speedup vs baseline: 1.1194x; 1.1194x over previous
"""Block-sparse position-wise FFN on Trainium2 (Bass/Tile), 8-core data-parallel.

Strategy:
  - Shard tokens (B*S = 36928) evenly across 8 cores: 4616 tokens/core.
    The FFN is pointwise over tokens and both (masked) weight matrices fit
    in SBUF, so data-parallel needs no collectives.
  - Host prep (free for HW time): apply the 8x8 block masks to W1/W2,
    pre-transpose all operands to the layouts the PE wants. x is
    host-transposed to [DIM, T] so the device never transposes anything.
  - Both layers stream TOKENS as the moving operand with fp32r weights
    stationary (fp32r matmuls self-load weights — no LDWEIGHTS bubble;
    1 cycle/column at N >= 256). Outputs are produced transposed
    ([feature, token]) and un-transposed on host:
      fc1: hT[m]   = gelu(W1mT[:,m].T @ xT + b1[m])   per 128-wide f-tile
      fc2: outT[g] = W2mT[:,g].T @ hT + b2[g]         per 128-wide d-tile
    PE work per 256-token segment: (24*6 + 6*24) matmuls of N=256
    at ~109 ns each; ACT does gelu+bias (fc1) and copy+bias (fc2).
"""

import sys
import types

import numpy as np

# concourse's axon trace path imports antenv.axon_hooks, which this image
# lacks; install a no-op shim so an env-requested trace degrades gracefully
# instead of raising ImportError.
try:
    import antenv.axon_hooks  # noqa: F401
except ImportError:
    import antenv

    _hooks = types.ModuleType("antenv.axon_hooks")
    _hooks._hook = None
    _hooks.set_axon_ntff_profile_hook = (
        lambda h: setattr(_hooks, "_hook", h))
    _hooks.get_axon_ntff_profile_hook = lambda: _hooks._hook
    sys.modules["antenv.axon_hooks"] = _hooks
    antenv.axon_hooks = _hooks

import concourse.bass as bass
import concourse.bacc as bacc
import concourse.mybir as mybir
from concourse import tile
from concourse.bass_utils import run_bass_kernel_spmd

B, S, DIM, FF, BLK = 64, 577, 768, 3072, 8
NCORES = 8
TOK = B * S                # 36928
T = TOK // NCORES          # 4616 tokens per core
P = 128
KD = DIM // P              # 6 d-tiles
KF = FF // P               # 24 f-tiles
SEG = 384                  # tokens per segment (fp32r full rate needs >=256)
F32 = mybir.dt.float32
F32R = mybir.dt.float32r
GELU = mybir.ActivationFunctionType.Gelu
IDENT = mybir.ActivationFunctionType.Identity


def _segments(total):
    """256-token segments; a trailing remainder <256 is folded into the last
    segment so every segment stays in [256, 512] (fp32r full rate, one PSUM
    bank)."""
    out, pos = [], 0
    while pos < total:
        rem = total - pos
        w = SEG if rem > 2 * SEG else rem
        out.append((pos, w))
        pos += w
    return out


SEGW = 392  # max segment width ([384]*11 + [392] for T=4616)


def _body(tc, x_d, w1_d, b1_d, w2_d, b2_d, o_d, t_tokens, K1, K2):
    nc = tc.nc
    with (
        tc.tile_pool(name="const", bufs=1) as constp,
        tc.tile_pool(name="wpool", bufs=1) as wp,
        tc.tile_pool(name="xt", bufs=2) as xtp,
        tc.tile_pool(name="ht", bufs=25) as htp,
        tc.tile_pool(name="onat", bufs=3) as onatp,
        tc.tile_pool(name="ps1", bufs=3, space=bass.MemorySpace.PSUM) as ps1p,
        tc.tile_pool(name="ps2", bufs=3, space=bass.MemorySpace.PSUM) as ps2p,
    ):
        b1_s = constp.tile([P, KF], F32)
        b2_s = constp.tile([P, KD], F32)

        # fc1 weights: w1_s[k] = W1mT[k*128:(k+1)*128, :]  ([128 d, 3072 f])
        # loaded in 8 column chunks so fc1 can start after the first chunk;
        # bias loads are deferred behind the first chunk (they gate only the
        # much-later ACTs, not the first matmuls).
        w1_s = [wp.tile([P, FF], F32R, tag=f"w1_{k}", name=f"w1_{k}")
                for k in range(KD)]
        W1CHUNK = FF // 8
        for cc in range(8):
            for k in range(KD):
                nc.sync.dma_start(
                    out=w1_s[k][:, cc * W1CHUNK:(cc + 1) * W1CHUNK],
                    in_=w1_d[k * P:(k + 1) * P,
                             cc * W1CHUNK:(cc + 1) * W1CHUNK],
                )
            if cc == 0:
                nc.sync.dma_start(out=b1_s[:], in_=b1_d)
                nc.sync.dma_start(out=b2_s[:], in_=b2_d)
        # fc2 weights: w2_s[k] = W2mT[k*128:(k+1)*128, :]  ([128 f, 768 d])
        w2_s = []
        for k in range(KF):
            w = wp.tile([P, DIM], F32R, tag=f"w2_{k}")
            nc.sync.dma_start(out=w[:], in_=w2_d[k * P:(k + 1) * P, :])
            w2_s.append(w)

        for (s0, w) in _segments(t_tokens):
            # x arrives pre-transposed from host: xT tile [128 d, w tokens]
            xts = []
            for k in range(KD):
                xt = xtp.tile([P, SEGW], F32R, tag=f"xt{k}", name=f"xt{k}")
                nc.gpsimd.dma_start(
                    out=xt[:, 0:w], in_=x_d[k * P:(k + 1) * P, s0:s0 + w]
                )
                xts.append(xt)

            # --- fc1: hT[m] = gelu(W1mT[:,m].T @ xT + b1[m]) ---
            hts = []
            for m in range(KF):
                ps = ps1p.tile([P, SEGW], F32, tag="ps1")
                ks = K1[m]
                for j, k in enumerate(ks):
                    nc.tensor.matmul(
                        ps[:, 0:w],
                        w1_s[k][:, m * P:(m + 1) * P],
                        xts[k][:, 0:w],
                        start=(j == 0), stop=(j == len(ks) - 1),
                    )
                ht = htp.tile([P, SEGW], F32R, tag="ht")
                nc.scalar.activation(
                    ht[:, 0:w], ps[:, 0:w], GELU, bias=b1_s[:, m:m + 1]
                )
                hts.append(ht)

            # --- fc2: outT[g] = W2mT[:,g].T @ hT + b2[g] ---
            for g in range(KD):
                ps = ps2p.tile([P, SEGW], F32, tag="ps2")
                ks2 = K2[g]
                for j, k in enumerate(ks2):
                    nc.tensor.matmul(
                        ps[:, 0:w],
                        w2_s[k][:, g * P:(g + 1) * P],
                        hts[k][:, 0:w],
                        start=(j == 0), stop=(j == len(ks2) - 1),
                    )
                on = onatp.tile([P, SEGW], F32, tag="on")
                nc.scalar.activation(
                    on[:, 0:w], ps[:, 0:w], IDENT, bias=b2_s[:, g:g + 1]
                )
                nc.sync.dma_start(
                    out=o_d[g * P:(g + 1) * P, s0:s0 + w], in_=on[:, 0:w]
                )


def build_program(K1, K2, t_tokens=T):
    nc = bacc.Bacc("TRN2", target_bir_lowering=False, debug=False,
                   num_devices=NCORES)
    x_d = nc.dram_tensor("xt", [DIM, t_tokens], F32R,
                         kind="ExternalInput").ap()
    w1_d = nc.dram_tensor("w1t", [DIM, FF], F32R, kind="ExternalInput").ap()
    b1_d = nc.dram_tensor("b1", [P, KF], F32, kind="ExternalInput").ap()
    w2_d = nc.dram_tensor("w2t", [FF, DIM], F32R, kind="ExternalInput").ap()
    b2_d = nc.dram_tensor("b2", [P, KD], F32, kind="ExternalInput").ap()
    o_d = nc.dram_tensor("out", [DIM, t_tokens], F32,
                         kind="ExternalOutput").ap()
    with tile.TileContext(nc) as tc:
        _body(tc, x_d, w1_d, b1_d, w2_d, b2_d, o_d, t_tokens, K1, K2)
    nc.compile()
    return nc


def _round_fp32r(a):
    """Round fp32 values to the fp32r grid (low 12 mantissa bits dropped,
    round-to-nearest), matching the PE's fp32r operand rounding."""
    u = a.view(np.uint32)
    u = (u + np.uint32(0x800)) & np.uint32(0xFFFFF000)
    return u.view(np.float32)


def host_prep(x, W1, b1, W2, b2, mask1, mask2, perm_f, perm_d):
    xt = _round_fp32r(np.ascontiguousarray(
        np.asarray(x, dtype=np.float32).reshape(TOK, DIM).T))  # [DIM, TOK]
    m1 = np.repeat(np.repeat(np.asarray(mask1, dtype=bool), BLK, 0), BLK, 1)
    m2 = np.repeat(np.repeat(np.asarray(mask2, dtype=bool), BLK, 0), BLK, 1)
    pf = (perm_f[:, None] * BLK + np.arange(BLK)[None, :]).ravel()
    pd = (perm_d[:, None] * BLK + np.arange(BLK)[None, :]).ravel()
    w1m = np.asarray(W1, np.float32) * m1.astype(np.float32)
    w1t = _round_fp32r(np.ascontiguousarray(w1m[pf].T))           # [DIM, FFp]
    w2m = np.asarray(W2, np.float32) * m2.astype(np.float32)
    w2t = _round_fp32r(np.ascontiguousarray(
        w2m[np.ix_(pd, pf)].T))                                   # [FFp, DIMp]
    b1h = np.ascontiguousarray(
        np.asarray(b1, np.float32)[pf].reshape(KF, P).T)          # [P, KF]
    b2h = np.ascontiguousarray(
        np.asarray(b2, np.float32)[pd].reshape(KD, P).T)          # [P, KD]
    return xt, w1t, b1h, w2t, b2h, pd


def _cluster(sup, tsz):
    """Greedy: build tiles of `tsz` blocks sharing a commonly-avoided window,
    so that tile's matmul chain can skip that window entirely."""
    nb, nwin = sup.shape
    unassigned = list(range(nb))
    tiles = []
    while True:
        best_w, best_av = None, None
        for wdw in range(nwin):
            av = [f for f in unassigned if not sup[f, wdw]]
            if len(av) >= tsz and (best_av is None or len(av) > len(best_av)):
                best_w, best_av = wdw, av
        if best_w is None:
            break
        best_av.sort(key=lambda f: int((~sup[f]).sum()))
        take = best_av[:tsz]
        tiles.append(take)
        for f in take:
            unassigned.remove(f)
    while unassigned:
        tiles.append(unassigned[:tsz])
        unassigned = unassigned[tsz:]
    perm = np.array([f for t in tiles for f in t])
    K = []
    for t in tiles:
        u = sup[t].any(0)
        ks = [wdw for wdw in range(nwin) if u[wdw]] or [0]
        K.append(ks)
    return perm, K


def _plan(mask1, mask2):
    m1 = np.asarray(mask1, dtype=bool)
    m2 = np.asarray(mask2, dtype=bool)
    sup1 = m1.reshape(FF // BLK, KD, P // BLK).any(2)
    perm_f, K1 = _cluster(sup1, P // BLK)
    sup2 = m2[:, perm_f].reshape(DIM // BLK, KF, P // BLK).any(2)
    perm_d, K2 = _cluster(sup2, P // BLK)
    return perm_f, K1, perm_d, K2


_PROGRAM = None
_PROGRAM_KEY = None


def _get_program(mask1, mask2):
    global _PROGRAM, _PROGRAM_KEY
    key = (np.asarray(mask1).tobytes(), np.asarray(mask2).tobytes())
    if _PROGRAM is None or _PROGRAM_KEY != key:
        perm_f, K1, perm_d, K2 = _plan(mask1, mask2)
        _PROGRAM = (build_program(K1, K2, T), perm_f, perm_d)
        _PROGRAM_KEY = key
    return _PROGRAM


def kernel(x, W1, b1, W2, b2, mask1, mask2, **run_kwargs):
    nc, perm_f, perm_d = _get_program(mask1, mask2)
    xt, w1t, b1h, w2t, b2h, pd = host_prep(
        x, W1, b1, W2, b2, mask1, mask2, perm_f, perm_d)
    in_maps = [
        {"xt": np.ascontiguousarray(xt[:, c * T:(c + 1) * T]),
         "w1t": w1t, "b1": b1h, "w2t": w2t, "b2": b2h}
        for c in range(NCORES)
    ]
    res = run_bass_kernel_spmd(nc, in_maps, list(range(NCORES)), **run_kwargs)
    out = np.concatenate(
        [res.results[c]["out"] for c in range(NCORES)], axis=1)  # [DIMp, TOK]
    inv = np.empty(DIM, np.int64)
    inv[pd] = np.arange(DIM)
    out = out[inv]
    out = np.ascontiguousarray(out.T).reshape(B, S, DIM).astype(np.float32)
    if run_kwargs:
        kernel.last_results = res
    return out
